# revision 31
# baseline (speedup 1.0000x reference)
"""Trainium2 Bass kernel for nn_CausalSelfAttention_38216619000057.

Reference semantics (faithful to the source bug q = k):
    qkv = x @ W_attn + b_attn ; _, k, v = split(qkv)
    S = (K K^T) * D**-0.5  (per head, causal-masked), P = softmax(S)
    out = (P V) reshaped @ W_proj + b_proj

Sharding over 8 cores: data-parallel on B (4), tensor-parallel on heads (2
groups of 8). Core c handles batch c//2, heads 8*(c%2)..8*(c%2)+7, and
produces a partial projection output; the host sums the two partials per
batch and adds b_proj + b_v @ W_proj (the V-bias contribution commutes
through softmax because rows of P sum to 1).

Layout: the per-core x shard is passed host-side as x^T in chunk-major
bf16 (xt[p, ci, eb, t'] = x[512*ci + t', 128*eb + p]) -- the same kind of
layout permutation applied to the weight shards -- so K^T = W_k^T X^T and
V = X W_v consume it directly and no on-chip transposes are needed.

Since q = k, S is symmetric, so S^T tiles (keys on partitions, queries on
the free axis) are computed directly from the K^T operand, which is also
what the P V matmul needs as its moving operand. K^T is written as fp8e4m3
scaled by 4*sqrt(D**-0.5) (bias folded in), then DMA-packed into the
DoubleRow layout (h = 2p + ko on 32 partitions per head), so each S^T
matmul runs at 0.5 cycles/row with both heads of a pair on distinct PE
row groups. The 16x scale inflation of S is compensated inside the Exp
activation (scale=1/16).

The causal mask is applied on the exp(S^T) tiles over the 128-wide
diagonal strip of each crossing block -- head q=0 via Pool affine_select
(fill 0), head q=1 via a DVE multiply with a precomputed 0/1 triangle,
so the two strips mask in parallel and the PV matmuls that gate on them
never serialize through one engine queue. No PE cost. The softmax denominators come free from a column of
ones appended to V (the ones-row of the PV accumulation sums the
surviving E entries); outputs are normalized with a
reciprocal/broadcast-matmul/multiply chain and projected per chunk.

Schedule: software-pipelined by 512-query chunk -- chunk ci+1's K/V
matmul pieces are emitted interleaved into chunk ci's attention streams
(causal rows of ci need only chunks 0..ci), so the Activation engine's
exp stream (the second-busiest engine) starts ~5us into the kernel and
stays fed while the PE alternates K/V, S (DoubleRow), and PV work;
each chunk's projection is deferred past the next chunk's first
attention stream so the S matmuls claim the shared tag-s PSUM slots
ahead of the projection burst at every chunk boundary.
Input DMAs are split across the two HWDGE queues (xt sliced on SP;
wk sliced per head-pair, then wv, wp on Activation; output stores
alternate queues) so the serial DMA pipe never gates the first matmuls.
PSUM: K/V accumulators (2 banks), S tiles (2x2 banks, shared with the
projection/broadcast tiles), PV accumulators (2 banks).

Further cuts on top of the 194us checkpoint: fully-masked query columns
of diagonal-crossing blocks (il < 128*oi has no valid keys) are skipped in
the S matmuls, the exp, and the PV accumulation -- the affine_select mask
shrinks to one 128-wide strip per crossing block; the K^T scale-bias copy
moved from the (busier) Activation engine to DVE; wk/wv/wp loads are
hoisted out of the rep body; and kt/kt_dr/v_ones are double-buffered by
rep parity so consecutive reps in one NEFF overlap instead of
serializing on the K-cache writes. E tiles and v_ones are bf16.

Measured (bench2.py, N-reps-in-one-NEFF wall-clock differencing on the 8
axon-tunneled cores, stable-window): 194401 ns/rep for the predecessor of
this kernel vs 386037 ns for the session-start baseline by the same
method; this version simulates ~6% faster still (288 vs 305 us in
TimelineSim) and was indistinguishable from the predecessor in a later,
~2x-degraded measurement window. Relative error vs the jax reference:
4.8e-3 (budget 2e-2; fp8 K^T quantization dominates, and its
softmax-weight jitter averages out over 2048 keys).
"""

import threading

import numpy as np

import concourse.bacc as bacc
import concourse.mybir as mybir
import concourse.tile as tile
from concourse.bass_utils import run_bass_kernel_spmd

B, T, D = 4, 2048, 1024
H = 16
HD = 64
NCORES = 8
HPC = 8  # heads per core
ISQ = float(D**-0.5) ** 0.5  # K is pre-scaled by sqrt(D**-0.5)
F32 = mybir.dt.float32
F32R = mybir.dt.float32r
BF16 = mybir.dt.bfloat16
FP8 = mybir.dt.float8e4

Ident = mybir.ActivationFunctionType.Identity
Exp = mybir.ActivationFunctionType.Exp
Mult = mybir.AluOpType.mult
IsGe = mybir.AluOpType.is_ge
DR = mybir.MatmulPerfMode.DoubleRow
KSC = 4.0  # fp8 K^T pre-scale; S comes out 16x, compensated in the exp

_cache_lock = threading.Lock()
_cached_nc = {}


def _declare_io(nc, synth=False):
    kind = "Internal" if synth else "ExternalInput"
    ts = {}
    ts["xt"] = nc.dram_tensor("xt", [128, 4, 8, 512], BF16, kind=kind)
    ts["wk"] = nc.dram_tensor("wk", [128, 4, 8, 128], BF16, kind=kind)
    ts["wv"] = nc.dram_tensor("wv", [128, 8, 512], BF16, kind=kind)
    ts["wp"] = nc.dram_tensor("wp", [128, 4, 1024], BF16, kind=kind)
    ts["bk"] = nc.dram_tensor("bk", [128, 4], F32, kind=kind)
    ts["out"] = nc.dram_tensor("out", [T, D], F32, kind="Internal" if synth else "ExternalOutput")
    if synth:
        ts["done"] = nc.dram_tensor("done", [1, 4], F32, kind="ExternalOutput")
    return ts


def _synth_init(nc, tc, io):
    """Fill the Internal input tensors with benign constants on device."""
    with tc.tile_pool(name="init", bufs=1) as pool:
        it = pool.tile([128, 4096], F32, name="init_t")
        nc.vector.memset(it[:], 0.01)
        itb = pool.tile([128, 4096], BF16, name="init_tb")
        nc.vector.memset(itb[:], 0.01)
        for ci in range(4):
            nc.sync.dma_start(
                io["xt"][:, ci],
                itb[:, 0 : 8 * 512].rearrange("p (a b) -> p a b", a=8),
            )
        nc.sync.dma_start(io["wk"][:], itb[:, 0 : 4 * 8 * 128].rearrange("p (a b c) -> p a b c", a=4, b=8))
        nc.sync.dma_start(io["wv"][:], itb[:, 0 : 8 * 512].rearrange("p (a b) -> p a b", a=8))
        nc.sync.dma_start(io["wp"][:], itb[:, 0 : 4 * 1024].rearrange("p (a b) -> p a b", a=4))
        nc.sync.dma_start(io["bk"][:], it[:, 0:4])


def _emit_body(nc, tc, io, g, par=0):
    """One full forward pass, software-pipelined by chunk.

    Chunk ci+1's K^T/V matmul pieces are emitted interleaved into chunk ci's
    attention stream (causal: rows of chunk ci need only chunks 0..ci), so
    the PE alternates K/V and S/PV work while the Activation engine streams
    exps continuously instead of idling through each K/V block.
    """
    kt_sb, kt_dr, v_ones = g["kt_sb"], g["kt_dr"], g["v_ones"]
    wk_sb, wv_sb = g["wk_sb"], g["wv_sb"]
    wp_sb, bk_sb = g["wp_sb"], g["bk_sb"]
    ones_sb = g["ones_sb"]
    tri_sb = g["tri_sb"]
    out = io["out"]

    with (
        tc.tile_pool(name="xtp", bufs=3) as xtp,
        tc.tile_pool(name="psKV", bufs=2, space="PSUM") as psKV,
        tc.tile_pool(name="ps_s", bufs=2, space="PSUM") as ps_s,
        tc.tile_pool(name="ps_pv", bufs=2, space="PSUM") as ps_pv,
        tc.tile_pool(name="ebuf", bufs=10) as ebuf,
        tc.tile_pool(name="rbuf", bufs=8) as rbuf,
        tc.tile_pool(name="obuf", bufs=4) as obuf,
        tc.tile_pool(name="obig", bufs=1) as obig,
    ):
        o_t = obig.tile([128, 4, T], BF16, name="o_t")
        if g.get("wp_pending"):
            # one-time weight loads, all on the Activation HWDGE queue (x
            # chunks use SP, so the first K matmuls aren't stuck behind
            # them). wk is sliced per head-pair so K(hp=0) starts after
            # ~0.25MB lands; wp (projection-only) queues last.
            for hp in range(4):
                nc.scalar.dma_start(wk_sb[:, hp], io["wk"][:, hp])
            nc.scalar.dma_start(wv_sb[:], io["wv"][:])
            nc.scalar.dma_start(wp_sb[:], io["wp"][:])
            g["wp_pending"] = False

        def load_xt(ci):
            xt_chunk = xtp.tile([128, 8, 512], BF16, tag="xtc", name="xt_chunk")
            # sliced along eb so the first K matmuls start after ~2 e-blocks
            for ebs in range(4):
                nc.sync.dma_start(
                    xt_chunk[:, 2 * ebs : 2 * ebs + 2, :],
                    io["xt"][:, ci, 2 * ebs : 2 * ebs + 2, :],
                )
            return xt_chunk

        def emit_k(ci, hp, xt_chunk):
            kps = psKV.tile([128, 512], F32, tag="kv", name="kps")
            for eb in range(8):
                nc.tensor.matmul(
                    kps[:],
                    wk_sb[:, hp, eb, :],
                    xt_chunk[:, eb, :],
                    start=(eb == 0),
                    stop=(eb == 7),
                )
            nc.vector.tensor_scalar(
                kt_sb[:, par, hp, ci * 512 : (ci + 1) * 512],
                kps[:],
                KSC * ISQ,
                bk_sb[:, hp : hp + 1],
                Mult,
                mybir.AluOpType.add,
            )
            # pack the two heads' 64 hd-rows as (p, ko) -> h = 2p + ko on
            # 32 partitions each, the DoubleRow layout the S matmuls need
            nc.scalar.dma_start(
                kt_dr[:, par, hp, :, ci * 512 : (ci + 1) * 512],
                kt_sb[:, par, hp, ci * 512 : (ci + 1) * 512],
            )

        def emit_v(ci, tbl, xt_chunk):
            vps = psKV.tile([128, 512], F32, tag="kv", name="vps")
            for eb in range(8):
                nc.tensor.matmul(
                    vps[:],
                    xt_chunk[:, eb, tbl * 128 : (tbl + 1) * 128],
                    wv_sb[:, eb, :],
                    start=(eb == 0),
                    stop=(eb == 7),
                )
            tb = 4 * ci + tbl
            nc.vector.tensor_copy(
                v_ones[:, par, tb, :].rearrange("p (h c) -> p h c", c=65)[:, :, 0:64],
                vps[:].rearrange("p (h c) -> p h c", c=64),
            )

        def emit_attn(ci, hp):
            njb = 4 * ci + 4
            rhs = [
                kt_dr[32 * q : 32 * q + 32, par, hp, :, ci * 512 : (ci + 1) * 512]
                for q in (0, 1)
            ]
            pv = [ps_pv.tile([65, 512], F32, tag="pv", name="pv") for _ in (0, 1)]
            for jbp in range(njb // 2):
                sps = [
                    ps_s.tile([128, 1024], F32, tag="s", name="sps") for _ in (0, 1)
                ]
                # crossing block jb has query offset oi = jb - 4*ci; its
                # first 128*oi columns have no valid keys at all (il < 128*oi
                # => il < p + 128*oi for every p), so S/exp/PV skip them.
                def voff(jb):
                    oi = jb - 4 * ci
                    return 128 * oi if 0 < oi <= 3 else 0

                for half in (0, 1):
                    jb = 2 * jbp + half
                    vo = voff(jb)
                    for q in (0, 1):  # adjacent MMs hit distinct row groups
                        nc.tensor.matmul(
                            sps[q][:, half * 512 + vo : half * 512 + 512],
                            kt_dr[32 * q : 32 * q + 32, par, hp, :, jb * 128 : (jb + 1) * 128],
                            rhs[q][:, :, vo:512],
                            start=True,
                            stop=True,
                            perf_mode=DR,
                        )
                vo0, vo1 = voff(2 * jbp), voff(2 * jbp + 1)
                eps = []
                for q in (0, 1):
                    ep = ebuf.tile([128, 1024], BF16, tag="e", name="ep")
                    if vo0 + vo1 >= 512:
                        # mostly-dead pair: exp the two valid spans separately
                        nc.scalar.activation(
                            ep[:, vo0:512], sps[q][:, vo0:512], Exp,
                            scale=1.0 / (KSC * KSC),
                        )
                        nc.scalar.activation(
                            ep[:, 512 + vo1 : 1024], sps[q][:, 512 + vo1 : 1024],
                            Exp, scale=1.0 / (KSC * KSC),
                        )
                    else:
                        nc.scalar.activation(
                            ep[:], sps[q][:], Exp, scale=1.0 / (KSC * KSC)
                        )
                    eps.append(ep)
                for half in (0, 1):
                    jb = 2 * jbp + half
                    if jb >= 4 * ci:
                        # partially-masked 128-col strip at the block diagonal:
                        # zero j > i (local il' < p) on Pool
                        oi = jb - 4 * ci
                        c0 = half * 512 + 128 * oi
                        # the two heads' mask strips run on different engines
                        # (q=0 Pool affine_select, q=1 DVE multiply by the
                        # precomputed 0/1 triangle) so they don't serialize
                        # through one queue -- the PV matmuls gate on these
                        nc.gpsimd.affine_select(
                            out=eps[0][:, c0 : c0 + 128],
                            in_=eps[0][:, c0 : c0 + 128],
                            pattern=[[1, 128]],
                            compare_op=IsGe,
                            fill=0.0,
                            base=0,
                            channel_multiplier=-1,
                        )
                        nc.vector.tensor_tensor(
                            eps[1][:, c0 : c0 + 128],
                            eps[1][:, c0 : c0 + 128],
                            tri_sb[:],
                            Mult,
                        )
                for half in (0, 1):
                    jb = 2 * jbp + half
                    vo = voff(jb)
                    for q in (0, 1):
                        hl = 2 * hp + q
                        nc.tensor.matmul(
                            pv[q][:, vo:512],
                            v_ones[:, par, jb, 65 * hl : 65 * hl + 65],
                            eps[q][:, half * 512 + vo : half * 512 + 512],
                            start=(jb == 0),
                            stop=(jb == njb - 1),
                        )
            for q in (0, 1):
                r_row = rbuf.tile([1, 512], F32R, tag="rr", name="r_row")
                with nc.allow_low_precision(
                    reason="f32r reciprocal output feeds bc matmul"
                ):
                    nc.vector.reciprocal(r_row[:], pv[q][64:65, :])
                bcps = ps_s.tile([64, 512], F32, tag="s", name="bcps")
                nc.tensor.matmul(bcps[:], ones_sb[:], r_row[:], start=True, stop=True)
                r_bc = rbuf.tile([64, 512], F32, tag="rb", name="r_bc")
                nc.vector.tensor_copy(r_bc[:], bcps[:])
                nc.vector.tensor_tensor(
                    o_t[64 * q : 64 * q + 64, hp, ci * 512 : (ci + 1) * 512],
                    pv[q][0:64, :],
                    r_bc[:],
                    Mult,
                )

        def emit_proj(ci):
            for tbl in range(4):
                tb = 4 * ci + tbl
                for nch in range(2):
                    ops_ = ps_s.tile([128, 512], F32, tag="s", name="ops")
                    for hp2 in range(4):
                        nc.tensor.matmul(
                            ops_[:],
                            o_t[:, hp2, tb * 128 : (tb + 1) * 128],
                            wp_sb[:, hp2, nch * 512 : (nch + 1) * 512],
                            start=(hp2 == 0),
                            stop=(hp2 == 3),
                        )
                    ob = obuf.tile([128, 512], F32, tag="ob", name="ob")
                    nc.vector.tensor_copy(ob[:], ops_[:])
                    dma_eng = nc.sync if nch == 0 else nc.scalar
                    dma_eng.dma_start(
                        out[tb * 128 : (tb + 1) * 128, nch * 512 : (nch + 1) * 512],
                        ob[:],
                    )

        # chunk 0's K/V up front, then pipeline: chunk ci+1's K/V pieces are
        # interleaved into chunk ci's attention loop
        xt_cur = load_xt(0)
        for hp in range(4):
            emit_k(0, hp, xt_cur)
        for tbl in range(4):
            emit_v(0, tbl, xt_cur)
        pending_proj = None
        for ci in range(4):
            xt_nxt = load_xt(ci + 1) if ci + 1 < 4 else None
            for hp in range(4):
                emit_attn(ci, hp)
                if hp == 0 and pending_proj is not None:
                    # chunk ci-1's projection is emitted AFTER chunk ci's
                    # first attention stream, so at the chunk boundary the
                    # S matmuls claim the shared tag-s PSUM slots first and
                    # the Activation exp stream never starves behind the
                    # projection burst. Dependencies are unchanged -- o_t of
                    # ci-1 is complete either way.
                    emit_proj(pending_proj)
                    pending_proj = None
                if xt_nxt is not None:
                    emit_k(ci + 1, hp, xt_nxt)
                    emit_v(ci + 1, hp, xt_nxt)
            pending_proj = ci
            xt_cur = xt_nxt
        emit_proj(3)

def _build_program(nreps: int = 1, synth: bool = False):
    nc = bacc.Bacc("TRN2", target_bir_lowering=False)
    io = _declare_io(nc, synth=synth)

    with tile.TileContext(nc) as tc:
        if synth:
            _synth_init(nc, tc, io)
        with tc.tile_pool(name="singles", bufs=1) as singles:
            g = {}
            g["kt_sb"] = singles.tile([128, 2, 4, T], FP8, name="kt_sb")
            g["kt_dr"] = singles.tile([64, 2, 4, 2, T], FP8, name="kt_dr")
            g["v_ones"] = singles.tile([128, 2, 16, HPC * 65], BF16, name="v_ones")
            g["wk_sb"] = singles.tile([128, 4, 8, 128], BF16, name="wk_sb")
            g["wv_sb"] = singles.tile([128, 8, 512], BF16, name="wv_sb")
            g["wp_sb"] = singles.tile([128, 4, 1024], BF16, name="wp_sb")
            g["bk_sb"] = singles.tile([128, 4], F32, name="bk_sb")
            g["ones_sb"] = singles.tile([1, 64], F32R, name="ones_sb")
            g["tri_sb"] = singles.tile([128, 128], BF16, name="tri_sb")

            nc.scalar.dma_start(g["bk_sb"][:], io["bk"][:])
            nc.vector.memset(g["ones_sb"][:].bitcast(F32), 1.0)
            nc.gpsimd.memset(g["tri_sb"][:], 1.0)
            nc.gpsimd.affine_select(
                out=g["tri_sb"][:],
                in_=g["tri_sb"][:],
                pattern=[[1, 128]],
                compare_op=IsGe,
                fill=0.0,
                base=0,
                channel_multiplier=-1,
            )
            nc.vector.memset(
                g["v_ones"][:]
                .rearrange("p r t (h c) -> p r t h c", c=65)[:, :, :, :, 64:65],
                1.0,
            )

            g["wp_pending"] = True
            for _rep in range(nreps):
                _emit_body(nc, tc, io, g, _rep % 2)

            if synth:
                with tc.tile_pool(name="fin", bufs=1) as fin:
                    dn = fin.tile([1, 4], F32, name="dn")
                    nc.vector.memset(dn[:], 1.0)
                    nc.sync.dma_start(io["done"][:], dn[:])

    nc.compile()
    return nc


def _build_null_program():
    """Same I/O signature, trivial body -- for wall-clock differencing."""
    nc = bacc.Bacc("TRN2", target_bir_lowering=False)
    io = _declare_io(nc)
    with tile.TileContext(nc) as tc:
        with tc.tile_pool(name="sb", bufs=2) as sb:
            t = sb.tile([128, 512], F32)
            nc.sync.dma_start(t[:, 0:256], io["xt"][:, 0, 0].bitcast(F32))
            for tb in range(16):
                for nch in range(2):
                    nc.sync.dma_start(
                        io["out"][
                            tb * 128 : (tb + 1) * 128, nch * 512 : (nch + 1) * 512
                        ],
                        t[:],
                    )
    nc.compile()
    return nc


def _get_program(nreps: int = 1, synth: bool = False):
    with _cache_lock:
        key = (nreps, synth)
        if key not in _cached_nc:
            _cached_nc[key] = _build_program(nreps, synth)
        return _cached_nc[key]


def _core_inputs(c, x, W_attn, b_attn):
    b = c // 2
    h0 = HPC * (c % 2)
    c0k = D + h0 * HD
    c0v = 2 * D + h0 * HD
    import ml_dtypes

    # xt[p, ci, eb, t'] = x[b][512*ci + t', 128*eb + p]
    xt_np = np.ascontiguousarray(
        x[b].reshape(4, 512, 8, 128).transpose(3, 0, 2, 1).astype(ml_dtypes.bfloat16)
    )
    wk_np = np.ascontiguousarray(
        W_attn[:, c0k : c0k + 512].reshape(8, 128, 4, 128).transpose(1, 2, 0, 3)
        .astype(ml_dtypes.bfloat16)
    )
    wv_np = np.ascontiguousarray(
        W_attn[:, c0v : c0v + 512].reshape(8, 128, 512).transpose(1, 0, 2)
        .astype(ml_dtypes.bfloat16)
    )
    bk_np = np.ascontiguousarray(b_attn[c0k : c0k + 512].reshape(4, 128).T * (4.0 * ISQ))
    return {
        "xt": xt_np,
        "wk": wk_np,
        "wv": wv_np,
        "bk": bk_np,
    }


def _core_wp(c, W_proj):
    h0 = HPC * (c % 2)
    r0 = h0 * HD
    import ml_dtypes

    return np.ascontiguousarray(
        W_proj[r0 : r0 + 512, :].reshape(4, 128, 1024).transpose(1, 0, 2)
        .astype(ml_dtypes.bfloat16)
    )


def kernel(x, W_attn, b_attn, W_proj, b_proj, **_unused):
    x = np.asarray(x, dtype=np.float32)
    W_attn = np.asarray(W_attn, dtype=np.float32)
    b_attn = np.asarray(b_attn, dtype=np.float32)
    W_proj = np.asarray(W_proj, dtype=np.float32)
    b_proj = np.asarray(b_proj, dtype=np.float32)

    nc = _get_program()
    in_maps = []
    for c in range(NCORES):
        m = _core_inputs(c, x, W_attn, b_attn)
        m["wp"] = _core_wp(c, W_proj)
        in_maps.append(m)

    res = run_bass_kernel_spmd(nc, in_maps, core_ids=list(range(NCORES)))

    bias_row = b_proj + b_attn[2 * D : 3 * D] @ W_proj
    out = np.empty((B, T, D), dtype=np.float32)
    for b in range(B):
        out[b] = res.results[2 * b]["out"] + res.results[2 * b + 1]["out"] + bias_row
    return out
